# revision 1
# baseline (speedup 1.0000x reference)
"""CSPN accelerate (3x3 per-pixel dynamic filter) on 8 Trainium2 NeuronCores.

out[b,0,h,w] = sum_{di,dj in 0..2} K[b,3*di+dj,h,w] * Xpad[b, h+di-1, w+dj-1]
with the center tap (di=dj=1) taking input0 instead of input.

Sharding: pure data-parallel over batch (16 samples -> 2 per core).

Per-core kernel layout (per sample, per 126-row output tile):
  - x tile [128, 642]: input rows [r-1, r+127) in partitions, zero-padded
    columns 0/641 -> the three dj (column) shifts are free-dim slices.
  - k tiles (one per di) [128, 3, 640]: tap rows loaded shifted by -di so
    the per-tap product k*x is computed aligned to *input* rows on DVE.
  - 3 matmuls vs constant shifted-identity matrices [128, 126] realign the
    row (di) shifts and accumulate all taps into PSUM on the Tensor engine.
This reads every DRAM byte once (plus a 2/126 halo) instead of re-reading
the input three times for the row shifts.
"""

import numpy as np

import concourse.bacc as bacc
import concourse.bass as bass
import concourse.mybir as mybir
import concourse.tile as tile
from concourse.bass_utils import run_bass_kernel_spmd

F32 = mybir.dt.float32

BS, KK, H, W = 16, 9, 480, 640
N_CORES = 8
BPC = BS // N_CORES          # samples per core
P = 128                      # SBUF partitions
TH = P - 2                   # output rows per tile (input tile covers TH+2 rows)
W_CHUNKS = ((0, 512), (512, 128))  # matmul moving-dim <= 512, one PSUM bank each


def build_module() -> bass.Bass:
    nc = bacc.Bacc()
    k_ext = nc.declare_dram_parameter("kern", [BPC, KK, H, W], F32, isOutput=False)
    x_ext = nc.declare_dram_parameter("x", [BPC, 1, H, W], F32, isOutput=False)
    x0_ext = nc.declare_dram_parameter("x0", [BPC, 1, H, W], F32, isOutput=False)
    out_ext = nc.declare_dram_parameter("out", [BPC, 1, H, W], F32, isOutput=True)

    # Shift matrices: M[di][g, h] = 1 iff g == h + di, so that
    # out[h] = sum_g M[g,h] * prod[g] picks input-row-aligned products back
    # into output rows.
    m_np = np.zeros((P, 3, TH), np.float32)
    for di in range(3):
        for h in range(TH):
            m_np[h + di, di, h] = 1.0
    m_dram = nc.inline_tensor(m_np, name="shiftm")

    row_tiles = [(r, min(TH, H - r)) for r in range(0, H, TH)]

    with tile.TileContext(nc) as tc:
        with (
            tc.tile_pool(name="consts", bufs=1) as cpool,
            tc.tile_pool(name="kpool", bufs=5) as kpool,
            tc.tile_pool(name="xpool", bufs=4) as xpool,
            tc.tile_pool(name="prodpool", bufs=4) as ppool,
            tc.tile_pool(name="opool", bufs=4) as opool,
            tc.tile_pool(name="psum", bufs=4, space="PSUM") as psumpool,
        ):
            mtile = cpool.tile([P, 3, TH], F32)
            nc.sync.dma_start(out=mtile[:], in_=m_dram[:])

            for b in range(BPC):
                for r, th in row_tiles:
                    _emit_tile(
                        nc, kpool, xpool, ppool, opool, psumpool, mtile,
                        k_ext, x_ext, x0_ext, out_ext, b, r, th,
                    )
    nc.finalize()
    return nc


def _emit_tile(nc, kpool, xpool, ppool, opool, psumpool, mtile,
               k_ext, x_ext, x0_ext, out_ext, b, r, th):
    # --- input tile: rows [r-1, r-1+P) of x, zero-padded columns at 0, 641
    xt = xpool.tile([P, W + 2], F32)
    lo = r - 1
    clo, chi = max(lo, 0), min(lo + P, H)
    # Engine APs must start at partition 0/32/64/96: pad memsets cover an
    # aligned 32-row block, the DMA load below overwrites the valid rows.
    # Memsets run on the (otherwise idle) Activation engine.
    nc.gpsimd.memset(xt[:, 0:1], 0.0)
    nc.gpsimd.memset(xt[:, W + 1:W + 2], 0.0)
    if clo > lo:  # top image edge: zero the padding row(s)
        nc.gpsimd.memset(xt[0:32, :], 0.0)
    if chi < lo + P:  # bottom image edge
        nc.gpsimd.memset(xt[96:P, :], 0.0)
    nc.sync.dma_start(out=xt[clo - lo:chi - lo, 1:W + 1], in_=x_ext[b, 0, clo:chi, :])

    # --- center-tap replacement input0: rows [r, r+th) -> partitions [1, th+1)
    x0t = xpool.tile([P, W], F32, tag="x0t")
    nc.gpsimd.memset(x0t[0:32, :], 0.0)
    nc.gpsimd.memset(x0t[96:P, :], 0.0)
    nc.sync.dma_start(out=x0t[1:th + 1, :], in_=x0_ext[b, 0, r:r + th, :])

    # --- kernel taps, one tile per tap (keeps per-consumer sem waits low),
    # rows shifted by -di: k rows [r-di, r-di+P)
    kts = []
    for di in range(3):
        klo = r - di
        kclo, kchi = max(klo, 0), min(klo + P, H)
        row = []
        for dj in range(3):
            kt = kpool.tile([P, W], F32, tag=f"kt{di}{dj}")
            if kclo > klo:
                nc.gpsimd.memset(kt[0:32, :], 0.0)
            if kchi < klo + P:
                nc.gpsimd.memset(kt[96:P, :], 0.0)
            nc.sync.dma_start(
                out=kt[kclo - klo:kchi - klo, :],
                in_=k_ext[b, 3 * di + dj, kclo:kchi, :],
            )
            row.append(kt)
        kts.append(row)

    # --- fences: 1-elem copies that absorb the x/x0 DMA sem waits on each
    # consuming engine, so the tensor_tensor ops carry few sync waits.
    fence = ppool.tile([1, 2], F32, tag="fence")
    nc.vector.tensor_copy(out=fence[:, 0:1], in_=xt[64:65, 1:2])
    nc.vector.tensor_copy(out=fence[:, 1:2], in_=x0t[64:65, 0:1])

    # --- per-di product sums on DVE, row-shift + tap-sum via PE into PSUM.
    # di=0,2: 3 mults + 2 adds, one shift-matmul pair per group.
    # di=1: per-tap shift-matmuls (same M_1) -- trades 2 DVE adds for 4 PE
    # matmuls, balancing DVE (the busiest engine) against PE headroom.
    psum_t = psumpool.tile([P, W], F32)
    for di in (0, 2):
        prod = ppool.tile([P, W], F32, tag="prod")
        tmp = ppool.tile([P, W], F32, tag="tmp")
        nc.vector.tensor_tensor(
            out=prod[:], in0=kts[di][0][:], in1=xt[:, 0:W], op=mybir.AluOpType.mult)
        nc.vector.tensor_tensor(
            out=tmp[:], in0=kts[di][1][:], in1=xt[:, 1:W + 1], op=mybir.AluOpType.mult)
        nc.vector.tensor_tensor(
            out=prod[:], in0=prod[:], in1=tmp[:], op=mybir.AluOpType.add)
        nc.vector.tensor_tensor(
            out=tmp[:], in0=kts[di][2][:], in1=xt[:, 2:W + 2], op=mybir.AluOpType.mult)
        nc.vector.tensor_tensor(
            out=prod[:], in0=prod[:], in1=tmp[:], op=mybir.AluOpType.add)
        for c0, cn in W_CHUNKS:
            nc.tensor.matmul(
                out=psum_t[:th, c0:c0 + cn],
                lhsT=mtile[:, di, 0:th],
                rhs=prod[:, c0:c0 + cn],
                start=(di == 0),
                stop=False,
            )
    for dj in range(3):
        in1 = (xt[:, 0:W], x0t[:], xt[:, 2:W + 2])[dj]
        tmp = ppool.tile([P, W], F32, tag="tmp")
        nc.vector.tensor_tensor(
            out=tmp[:], in0=kts[1][dj][:], in1=in1, op=mybir.AluOpType.mult)
        for c0, cn in W_CHUNKS:
            nc.tensor.matmul(
                out=psum_t[:th, c0:c0 + cn],
                lhsT=mtile[:, 1, 0:th],
                rhs=tmp[:, c0:c0 + cn],
                start=False,
                stop=(dj == 2),
            )

    # --- PSUM -> SBUF (on ACT) -> DRAM
    osb = opool.tile([P, W], F32, tag="osb")
    nc.scalar.copy(out=osb[:th, :], in_=psum_t[:th, :])
    nc.scalar.dma_start(out=out_ext[b, 0, r:r + th, :], in_=osb[:th, :])


_NC_CACHE = None


def _get_module():
    global _NC_CACHE
    if _NC_CACHE is None:
        _NC_CACHE = build_module()
    return _NC_CACHE


def kernel(**inputs: np.ndarray) -> np.ndarray:
    kern = np.ascontiguousarray(np.asarray(inputs["kernel"], dtype=np.float32))
    x = np.ascontiguousarray(np.asarray(inputs["input"], dtype=np.float32))
    x0 = np.ascontiguousarray(np.asarray(inputs["input0"], dtype=np.float32))
    assert kern.shape == (BS, KK, H, W), kern.shape

    nc = _get_module()
    in_maps = [
        {
            "kern": np.ascontiguousarray(kern[c * BPC:(c + 1) * BPC]),
            "x": np.ascontiguousarray(x[c * BPC:(c + 1) * BPC]),
            "x0": np.ascontiguousarray(x0[c * BPC:(c + 1) * BPC]),
        }
        for c in range(N_CORES)
    ]
    res = run_bass_kernel_spmd(nc, in_maps, list(range(N_CORES)))
    out = np.concatenate([res.results[c]["out"] for c in range(N_CORES)], axis=0)
    return out.astype(np.float32)

